# revision 3
# baseline (speedup 1.0000x reference)
"""Trainium2 Bass kernel for nn_ChannelWiseConv (depthwise conv stack + KAN head).

v2 vs v1 (same conv1 front end, which already runs at the dispatch floor):
  - All post-conv bias+relu ops on the Activation engine only (each x2/x3/x5
    tile has a single writer engine -> no multi-writer wait-splitting).
  - conv4/5/6 and KAN PSUM tiles double-buffered so the two batch halves and
    consecutive KAN stages overlap instead of serializing on one bank.
  - KAN head: the D matrix (u - k replicated over basis offsets) is built by
    ONE PE matmul against a host-packed selector (replaces 12 DVE ops + a
    transpose per half), degree-0 basis via an is_ge difference (indicator
    subtraction), Cox-de-Boor levels entirely on DVE (gpsimd removed from
    the critical path), layer-2 silu taken from the SBUF copy of u2 so each
    PSUM tile keeps a single reader.
  - Cross-engine handoffs cost ~1.5-3us each on this relay when exposed
    (measured with synthetic probes); same-engine chains are ~free. The
    changes above cut the exposed-handoff count in the back half.
"""

import numpy as np

IN_CH, HIDDEN, NCLS = 13, 20, 10
B_FULL, NCORE = 2048, 8
B = B_FULL // NCORE          # images per core
NG = 16                      # image groups per core
GI = B // NG                 # images per group (16)
H_GRID = 0.4                 # KAN knot spacing; u = (x + 2.2) / 0.4

CG1 = [(0, 2), (2, 2), (4, 2), (6, 2), (8, 2), (10, 2), (12, 1)]
CG2 = [(0, 4), (4, 4), (8, 4), (12, 1)]
CG3 = [(0, 8), (8, 5)]

_BUILT = None  # cached (nc, input_names)


# ----------------------------------------------------------------------------
# host-side constant packing
# ----------------------------------------------------------------------------

def _pack_bands(w, S, cgs, slotM, rows):
    So = S // 2
    out = np.zeros((rows, len(cgs) * 3 * slotM), np.float32)
    for gi, (c0, nch) in enumerate(cgs):
        for b in range(3):
            col0 = (gi * 3 + b) * slotM
            for cl in range(nch):
                for i in range(So):
                    for a in range(3):
                        r = 2 * i + a - 1
                        if 0 <= r < S:
                            out[cl * S + r, col0 + cl * So + i] = w[c0 + cl, 0, a, b]
    return out


def _pack_bias_merged(bias, cgs, So, rows, per_col):
    ncol = (len(cgs) + per_col - 1) // per_col
    out = np.zeros((rows, ncol), np.float32)
    for gi, (c0, nch) in enumerate(cgs):
        col = gi // per_col
        p0 = (gi % per_col) * (cgs[0][1] * So)
        for cl in range(nch):
            out[p0 + cl * So:p0 + (cl + 1) * So, col] = bias[c0 + cl]
    return out


def _sel_D(nch):
    """Selector [nch+1, 12*nch]: col (k*nch+ch) -> row ch = 1, row nch = -k,
    so D = [u; 1].T @ S gives D[img, k*nch+ch] = u[ch, img] - k."""
    S = np.zeros((nch + 1, 12 * nch), np.float32)
    for k in range(12):
        for ch in range(nch):
            S[ch, k * nch + ch] = 1.0
            S[nch, k * nch + ch] = -float(k)
    return S


def _host_consts(inp):
    import ml_dtypes
    bf16 = ml_dtypes.bfloat16
    fp8 = ml_dtypes.float8_e4m3

    c = {}
    c["bands1"] = _pack_bands(inp["w1"], 64, CG1, 64, 128).astype(fp8)
    c["bands2"] = _pack_bands(inp["w2"], 32, CG2, 64, 128).astype(bf16)
    c["bands3"] = _pack_bands(inp["w3"], 16, CG3, 64, 128).astype(bf16)
    c["bands4"] = _pack_bands(inp["w4"], 8, [(0, 13)], 52, 104).astype(bf16)
    c["bands5"] = _pack_bands(inp["w5"], 4, [(0, 13)], 26, 52).astype(bf16)
    b6m = np.zeros((26, 26), np.float32)
    for bb in range(2):
        for ch in range(13):
            for r in range(2):
                b6m[ch * 2 + r, bb * 13 + ch] = inp["w6"][ch, 0, r, bb]
    c["bands6"] = b6m.astype(bf16)
    c["bv1"] = _pack_bias_merged(inp["b1"], CG1, 32, 128, 2)   # [128, 4]
    c["bv2"] = _pack_bias_merged(inp["b2"], CG2, 16, 128, 2)   # [128, 2]
    c["bv3"] = _pack_bias_merged(inp["b3"], CG3, 8, 104, 2)    # [104, 1]
    c["bv4"] = _pack_bias_merged(inp["b4"], [(0, 13)], 4, 52, 1)
    c["bv5"] = _pack_bias_merged(inp["b5"], [(0, 13)], 2, 26, 1)
    c["ubias"] = (2.5 * (inp["b6"] + 2.2)).astype(np.float32).reshape(13, 1)

    c["SD1"] = _sel_D(IN_CH)      # [14, 156]
    c["SD2"] = _sel_D(HIDDEN)     # [21, 240]

    # KAN layer 1 rhs pieces: silu part [13,20]; basis+bias part [105,20]
    c["c1a"] = np.ascontiguousarray(inp["sb1"].astype(np.float32))
    c1b = np.zeros((105, HIDDEN), np.float32)
    for n in range(8):
        for i in range(13):
            c1b[n * 13 + i] = inp["coef1"][i, :, n] * inp["ss1"][i] / 6.0
    c1b[104] = inp["bias1"]
    c["c1b"] = c1b
    # KAN layer 2 rhs pieces: silu [20,10]; basis rows 0..107; rows 108..159+bias
    c["c2s"] = np.ascontiguousarray(inp["sb2"].astype(np.float32))
    c2b = np.zeros((161, NCLS), np.float32)
    for n in range(8):
        for i in range(20):
            c2b[n * 20 + i] = inp["coef2"][i, :, n] * inp["ss2"][i] / 6.0
    c2b[160] = inp["bias2"]
    c["c2b1"] = np.ascontiguousarray(c2b[0:108])
    c["c2b2"] = np.ascontiguousarray(c2b[108:161])
    misc = np.zeros((128, 3), np.float32)
    misc[:, 0] = -2.2
    misc[:, 1] = 5.5
    c["misc"] = misc
    c["iden"] = np.eye(128, dtype=np.float32)
    return c


def _shard_x(x_shard):
    import ml_dtypes
    xs = x_shard.reshape(NG, GI, 13, 64, 64)
    xc = np.zeros((NG, 128, 7, GI, 64), np.float32)
    xa = xs[:, :, 0:12].transpose(0, 2, 3, 1, 4).reshape(NG, 6, 128, GI, 64)
    for cg in range(6):
        xc[:, :, cg] = xa[:, cg]
    xc[:, 0:64, 6] = xs[:, :, 12].transpose(0, 2, 1, 3)
    return np.ascontiguousarray(xc.astype(ml_dtypes.float8_e4m3))


# ----------------------------------------------------------------------------
# bass program
# ----------------------------------------------------------------------------

def _build():
    global _BUILT
    if _BUILT is not None:
        return _BUILT
    from contextlib import ExitStack
    import concourse.bass as bass  # noqa: F401
    import concourse.bacc as bacc
    import concourse.tile as tile
    import concourse.mybir as mybir

    f32 = mybir.dt.float32
    bf16 = mybir.dt.bfloat16
    fp8 = mybir.dt.float8e4
    AF = mybir.ActivationFunctionType
    OP = mybir.AluOpType
    AX = mybir.AxisListType

    nc = bacc.Bacc("TRN2")
    T = nc.tensor

    d_xc = nc.dram_tensor("xc", [NG, 128, 7, GI, 64], fp8, kind="ExternalInput")
    cons_shapes = {
        "bands1": ([128, 21 * 64], fp8), "bands2": ([128, 12 * 64], bf16),
        "bands3": ([128, 6 * 64], bf16), "bands4": ([104, 3 * 52], bf16),
        "bands5": ([52, 3 * 26], bf16), "bands6": ([26, 26], bf16),
        "bv1": ([128, 4], f32), "bv2": ([128, 2], f32), "bv3": ([104, 1], f32),
        "bv4": ([52, 1], f32), "bv5": ([26, 1], f32), "ubias": ([13, 1], f32),
        "misc": ([128, 3], f32), "c1a": ([13, 20], f32), "c1b": ([105, 20], f32),
        "c2s": ([20, 10], f32), "c2b1": ([108, 10], f32), "c2b2": ([53, 10], f32),
        "iden": ([128, 128], f32),
        "SD1": ([14, 156], f32), "SD2": ([21, 240], f32),
    }
    d_cons = {k: nc.dram_tensor(k, shp, dt, kind="ExternalInput")
              for k, (shp, dt) in cons_shapes.items()}
    d_out = nc.dram_tensor("out", [B, NCLS], f32, kind="ExternalOutput")

    with tile.TileContext(nc) as tc, ExitStack() as ctx:
        cpool = ctx.enter_context(tc.tile_pool(name="consts", bufs=1))
        tcons = {}
        early = ("bands1", "bands2", "bands3", "bv1", "bv2", "bv3")
        for k, (shp, dt) in cons_shapes.items():
            t = cpool.tile(shp, dt, name=f"t_{k}")
            if k in early:
                nc.scalar.dma_start(t[:, :], d_cons[k][:, :])
            else:
                nc.gpsimd.dma_start(t[:, :], d_cons[k][:, :])
            tcons[k] = t
        t_u = cpool.tile([14, B], f32, name="t_u")
        nc.vector.memset(t_u[:, :], 1.0)   # row 13 stays 1; rows 0:13 overwritten

        p_x1 = ctx.enter_context(tc.tile_pool(name="x1", bufs=3))
        p_x2 = ctx.enter_context(tc.tile_pool(name="x2", bufs=2))
        p_x3 = ctx.enter_context(tc.tile_pool(name="x3", bufs=2))
        p_sm = ctx.enter_context(tc.tile_pool(name="xsm", bufs=2))

        with tc.tile_pool(name="ps1", bufs=2, space="PSUM") as pp1, \
             tc.tile_pool(name="ps2", bufs=2, space="PSUM") as pp2, \
             tc.tile_pool(name="ps3456", bufs=2, space="PSUM") as pp3, \
             tc.tile_pool(name="kanps", bufs=1, space="PSUM") as ppk:
            scrap = pp1.tile([1, 16], f32, tag="scrap", bufs=1, name="scrap")
            for k in ("bands1", "bands2", "bands3"):
                T.matmul(scrap[0:1, 0:1], tcons[k][0:1, 0:1],
                         tcons[k][0:1, 0:1], start=True, stop=True)
            kpool = ctx.enter_context(tc.tile_pool(name="kan", bufs=2))

            def kan_half(t):
                sl = slice(t * 128, (t + 1) * 128)
                # ---- D = u - k for all (k, ch) via one matmul ----
                psD = ppk.tile([128, 256], f32, tag="kan", name="psD")
                T.matmul(psD[0:128, 0:156], t_u[0:14, sl], tcons["SD1"][:, :],
                         start=True, stop=True)
                D = kpool.tile([128, 156], f32, tag="D", name="Dt")
                nc.vector.tensor_copy(D[:, :], psD[0:128, 0:156])
                ge = kpool.tile([128, 156], f32, tag="ge", name="ge")
                nc.vector.tensor_scalar(ge[:, :], D[:, :], 0.0, None,
                                        op0=OP.is_ge)
                Bc = kpool.tile([128, 143], f32, tag="B0", name="Bc")
                nc.vector.tensor_sub(Bc[:, :], ge[:, 0:143], ge[:, 13:156])
                wid = 143
                for p in range(1, 4):
                    wid -= 13
                    ta = kpool.tile([128, wid], f32, tag=f"ta{p}", name="ta")
                    tb = kpool.tile([128, wid], f32, tag=f"tb{p}", name="tb")
                    nc.vector.tensor_mul(ta[:, :], D[:, 0:wid], Bc[:, 0:wid])
                    nc.vector.tensor_mul(tb[:, :],
                                         D[:, 13 * (p + 1):13 * (p + 1) + wid],
                                         Bc[:, 13:13 + wid])
                    if p < 3:
                        Bc = kpool.tile([128, wid], f32, tag=f"B{p}", name="Bc")
                        nc.vector.tensor_sub(Bc[:, :], ta[:, :], tb[:, :])
                    else:
                        Bc = kpool.tile([128, 105], f32, tag="B3", name="Bc")
                        nc.vector.tensor_sub(Bc[:, 0:104], ta[:, :], tb[:, :])
                        nc.vector.memset(Bc[:, 104:105], 1.0)
                # stacks: silu part [13,128]; (basis;1)^T part [105,128]
                stkA = kpool.tile([13, 128], f32, tag="stkA", name="stkA")
                stkB = kpool.tile([105, 128], f32, tag="stkB", name="stkB")
                ps_b1 = ppk.tile([128, 256], f32, tag="kan", name="ps_b1")
                T.transpose(ps_b1[0:105, 0:128], Bc[:, 0:105],
                            tcons["iden"][:, :])
                nc.vector.tensor_copy(stkB[:, :], ps_b1[0:105, 0:128])
                nc.scalar.activation(stkA[:, :], t_u[0:13, sl], AF.Silu,
                                     bias=tcons["misc"][0:13, 0:1], scale=H_GRID)
                ps_h1 = ppk.tile([128, 256], f32, tag="kan", name="ps_h1")
                T.matmul(ps_h1[0:128, 0:20], stkA[:, :], tcons["c1a"][:, :],
                         start=True, stop=False)
                T.matmul(ps_h1[0:128, 0:20], stkB[:, :], tcons["c1b"][:, :],
                         start=False, stop=True)
                # ---- KAN layer 2 ----
                u2 = kpool.tile([128, 20], f32, tag="u2", name="u2")
                nc.scalar.activation(u2[:, :], ps_h1[0:128, 0:20], AF.Identity,
                                     bias=tcons["misc"][0:128, 1:2], scale=2.5)
                # u2T with a ones row: memset whole, overwrite rows 0:20
                u2t = kpool.tile([21, 128], f32, tag="u2t", name="u2t")
                nc.vector.memset(u2t[:, :], 1.0)
                ps_t2 = ppk.tile([128, 256], f32, tag="kan", name="ps_t2")
                T.transpose(ps_t2[0:20, 0:128], u2[:, :], tcons["iden"][:, :])
                nc.vector.tensor_copy(u2t[0:20, :], ps_t2[0:20, 0:128])
                stk2s = kpool.tile([20, 128], f32, tag="s2s", name="stk2s")
                nc.scalar.activation(stk2s[:, :], u2t[0:20, :], AF.Silu,
                                     bias=tcons["misc"][0:20, 0:1], scale=H_GRID)
                psD2 = ppk.tile([128, 256], f32, tag="kan", name="psD2")
                T.matmul(psD2[0:128, 0:240], u2t[0:21, :], tcons["SD2"][:, :],
                         start=True, stop=True)
                D2 = kpool.tile([128, 240], f32, tag="D2", name="D2t")
                nc.vector.tensor_copy(D2[:, :], psD2[0:128, 0:240])
                ge2 = kpool.tile([128, 240], f32, tag="ge2", name="ge2")
                nc.vector.tensor_scalar(ge2[:, :], D2[:, :], 0.0, None,
                                        op0=OP.is_ge)
                Bc2 = kpool.tile([128, 220], f32, tag="B0_2", name="Bc2")
                nc.vector.tensor_sub(Bc2[:, :], ge2[:, 0:220], ge2[:, 20:240])
                wid = 220
                for p in range(1, 4):
                    wid -= 20
                    ta = kpool.tile([128, wid], f32, tag=f"t2a{p}", name="ta2")
                    tb = kpool.tile([128, wid], f32, tag=f"t2b{p}", name="tb2")
                    nc.vector.tensor_mul(ta[:, :], D2[:, 0:wid], Bc2[:, 0:wid])
                    nc.vector.tensor_mul(tb[:, :],
                                         D2[:, 20 * (p + 1):20 * (p + 1) + wid],
                                         Bc2[:, 20:20 + wid])
                    if p < 3:
                        Bc2 = kpool.tile([128, wid], f32, tag=f"B{p}_2",
                                         name="Bc2")
                        nc.vector.tensor_sub(Bc2[:, :], ta[:, :], tb[:, :])
                    else:
                        Bc2 = kpool.tile([128, 161], f32, tag="B3_2",
                                         name="Bc2")
                        nc.vector.tensor_sub(Bc2[:, 0:160], ta[:, :], tb[:, :])
                        nc.vector.memset(Bc2[:, 160:161], 1.0)
                stk2a = kpool.tile([108, 128], f32, tag="s2a", name="stk2a")
                stk2b = kpool.tile([53, 128], f32, tag="s2b", name="stk2b")
                ps_b2 = ppk.tile([128, 256], f32, tag="kan", name="ps_b2")
                T.transpose(ps_b2[0:108, 0:128], Bc2[:, 0:108],
                            tcons["iden"][:, :])
                nc.vector.tensor_copy(stk2a[:, :], ps_b2[0:108, 0:128])
                ps_b3 = ppk.tile([128, 256], f32, tag="kan", name="ps_b3")
                T.transpose(ps_b3[0:53, 0:128], Bc2[:, 108:161],
                            tcons["iden"][:, :])
                nc.vector.tensor_copy(stk2b[:, :], ps_b3[0:53, 0:128])
                ps_lg = ppk.tile([128, 256], f32, tag="kan", name="ps_lg")
                T.matmul(ps_lg[0:128, 0:NCLS], stk2a[:, :], tcons["c2b1"][:, :],
                         start=True, stop=False)
                T.matmul(ps_lg[0:128, 0:NCLS], stk2s[:, :], tcons["c2s"][:, :],
                         start=False, stop=False)
                T.matmul(ps_lg[0:128, 0:NCLS], stk2b[:, :], tcons["c2b2"][:, :],
                         start=False, stop=True)
                # ---- log_softmax (on an SBUF copy; ps_lg keeps 1 reader) ----
                lg_s = kpool.tile([128, NCLS], f32, tag="lg_s", name="lg_s")
                nc.vector.tensor_copy(lg_s[:, :], ps_lg[0:128, 0:NCLS])
                negm = kpool.tile([128, 1], f32, tag="negm", name="negm")
                nc.vector.reduce_max(negm[:, :], lg_s[:, :], axis=AX.X,
                                     negate=True)
                ex = kpool.tile([128, NCLS], f32, tag="ex", name="ex")
                nc.scalar.activation(ex[:, :], lg_s[:, :], AF.Exp,
                                     bias=negm[:, 0:1])
                ssum = kpool.tile([128, 1], f32, tag="ssum", name="ssum")
                nc.vector.reduce_sum(ssum[:, :], ex[:, :], axis=AX.X)
                lsum = kpool.tile([128, 1], f32, tag="lsum", name="lsum")
                nc.scalar.activation(lsum[:, :], ssum[:, :], AF.Ln,
                                     bias=tcons["misc"][0:128, 2:3])
                res = kpool.tile([128, NCLS], f32, tag="res", name="res")
                nc.vector.tensor_scalar(res[:, :], lg_s[:, :],
                                        negm[:, 0:1], lsum[:, 0:1],
                                        op0=OP.add, op1=OP.subtract)
                nc.sync.dma_start(d_out[sl, :], res[:, :])

            x2t = x3t = x4 = x5 = None
            for g in range(NG):
                xt = p_x1.tile([128, 7 * GI * 64], fp8, tag="x1", name="xt")
                nc.sync.dma_start(
                    xt[:, :].rearrange("p (s i w) -> p s i w", i=GI, w=64),
                    d_xc[g, :, :, :, :])
                T.matmul(scrap[0:1, 0:1], xt[0:1, 0:1], xt[0:1, 0:1],
                         start=True, stop=True)
                # ---- conv1 ----
                if g % 2 == 0:
                    x2t = [p_x2.tile([128, 2 * GI * 32], bf16, tag=f"x2_{k}",
                                     name=f"x2_{k}") for k in range(4)]
                c0_, c1_ = (g % 2) * GI * 32, (g % 2 + 1) * GI * 32
                ps_pair = None
                for cg, (c0, nch) in enumerate(CG1):
                    K, M = nch * 64, nch * 32
                    xv = xt[0:K, cg * GI * 64:(cg + 1) * GI * 64].rearrange(
                        "p (i w) -> p i w", w=64)
                    if cg % 2 == 0:
                        ps_pair = pp1.tile([128, GI * 32], f32, tag="ps1",
                                           name="ps1t")
                    po = 64 * (cg % 2)
                    ps = ps_pair[po:po + M, :]
                    pv = ps.rearrange("p (i w) -> p i w", w=32)
                    lo = lambda b: (cg * 3 + b) * 64
                    T.matmul(pv, tcons["bands1"][0:K, lo(1):lo(1) + M],
                             xv[:, :, 0:64:2], start=True, stop=False,
                             skip_group_check=True)
                    T.matmul(pv, tcons["bands1"][0:K, lo(2):lo(2) + M],
                             xv[:, :, 1:64:2], start=False, stop=False,
                             skip_group_check=True)
                    T.matmul(pv[:, :, 1:32], tcons["bands1"][0:K, lo(0):lo(0) + M],
                             xv[:, :, 1:62:2], start=False, stop=True,
                             skip_group_check=True)
                    if cg % 2 == 1 or cg == 6:
                        pair = cg // 2
                        Mt = 128 if cg != 6 else 32
                        nc.scalar.activation(x2t[pair][0:Mt, c0_:c1_],
                                             ps_pair[0:Mt, :], AF.Relu,
                                             bias=tcons["bv1"][0:Mt, pair:pair + 1])
                if g == 1:
                    for k in ("bands4", "bands5", "bands6", "c1a", "c1b",
                              "c2s", "c2b1", "c2b2", "iden", "SD1", "SD2",
                              "misc"):
                        T.matmul(scrap[0:1, 0:1], tcons[k][0:1, 0:1],
                                 tcons[k][0:1, 0:1], start=True, stop=True)
                if g % 2 != 1:
                    continue

                # ---- conv2 over a 2-group batch ----
                b2 = g // 2
                if b2 % 2 == 0:
                    x3t = [p_x3.tile([128, 4 * GI * 16], bf16, tag=f"x3_{k}",
                                     name=f"x3_{k}") for k in range(2)]
                d0, d1 = (b2 % 2) * 512, (b2 % 2 + 1) * 512
                ps_pair = None
                for k4, (c0, nch) in enumerate(CG2):
                    K, M = nch * 32, nch * 16
                    xv = x2t[k4][0:K, :].rearrange("p (i w) -> p i w", w=32)
                    if k4 % 2 == 0:
                        ps_pair = pp2.tile([128, 2 * GI * 16], f32, tag="ps2",
                                           name="ps2t")
                    po = 64 * (k4 % 2)
                    ps = ps_pair[po:po + M, :]
                    pv = ps.rearrange("p (i w) -> p i w", w=16)
                    lo = lambda b: (k4 * 3 + b) * 64
                    T.matmul(pv, tcons["bands2"][0:K, lo(1):lo(1) + M],
                             xv[:, :, 0:32:2], start=True, stop=False,
                             skip_group_check=True)
                    T.matmul(pv, tcons["bands2"][0:K, lo(2):lo(2) + M],
                             xv[:, :, 1:32:2], start=False, stop=False,
                             skip_group_check=True)
                    T.matmul(pv[:, :, 1:16], tcons["bands2"][0:K, lo(0):lo(0) + M],
                             xv[:, :, 1:30:2], start=False, stop=True,
                             skip_group_check=True)
                    if k4 % 2 == 1:
                        pair = k4 // 2
                        Mt = 128 if pair == 0 else 80
                        nc.scalar.activation(x3t[pair][0:Mt, d0:d1],
                                             ps_pair[0:Mt, :], AF.Relu,
                                             bias=tcons["bv2"][0:Mt, pair:pair + 1])
                if b2 % 2 != 1:
                    continue

                # ---- conv3 over a 4-group batch ----
                b3 = b2 // 2
                if b3 % 2 == 0:
                    x4 = p_sm.tile([104, 8 * GI * 8], bf16, tag="x4", name="x4")
                e0, e1 = (b3 % 2) * 512, (b3 % 2 + 1) * 512
                ps3 = pp3.tile([104, 4 * GI * 8], f32, tag="ps3", name="ps3t")
                for k8, (c0, nch) in enumerate(CG3):
                    K, M = nch * 16, nch * 8
                    xv = x3t[k8][0:K, :].rearrange("p (i w) -> p i w", w=16)
                    ps = ps3[64 * k8:64 * k8 + M, :]
                    pv = ps.rearrange("p (i w) -> p i w", w=8)
                    lo = lambda b: (k8 * 3 + b) * 64
                    T.matmul(pv, tcons["bands3"][0:K, lo(1):lo(1) + M],
                             xv[:, :, 0:16:2], start=True, stop=False,
                             skip_group_check=True)
                    T.matmul(pv, tcons["bands3"][0:K, lo(2):lo(2) + M],
                             xv[:, :, 1:16:2], start=False, stop=False,
                             skip_group_check=True)
                    T.matmul(pv[:, :, 1:8], tcons["bands3"][0:K, lo(0):lo(0) + M],
                             xv[:, :, 1:14:2], start=False, stop=True,
                             skip_group_check=True)
                nc.scalar.activation(x4[:, e0:e1], ps3[0:104, :], AF.Relu,
                                     bias=tcons["bv3"][0:104, 0:1])
                if b3 % 2 != 1:
                    continue

                # ---- conv4 over an 8-group batch ----
                b4 = b3 // 2
                x5 = p_sm.tile([52, 512], bf16, tag="x5", name="x5")
                xv = x4[0:104, :].rearrange("p (i w) -> p i w", w=8)
                ps4t = pp3.tile([104, 4 * GI * 8], f32, tag="ps3", name="ps4t")
                ps4 = ps4t[0:52, :]
                pv = ps4.rearrange("p (i w) -> p i w", w=4)
                T.matmul(pv, tcons["bands4"][0:104, 52:104], xv[:, :, 0:8:2],
                         start=True, stop=False)
                T.matmul(pv, tcons["bands4"][0:104, 104:156], xv[:, :, 1:8:2],
                         start=False, stop=False)
                T.matmul(pv[:, :, 1:4], tcons["bands4"][0:104, 0:52],
                         xv[:, :, 1:6:2], start=False, stop=True,
                         skip_group_check=True)
                nc.scalar.activation(x5[:, :], ps4[:, :], AF.Relu,
                                     bias=tcons["bv4"][0:52, 0:1])

                # ---- conv5 ----
                x6 = p_sm.tile([26, 256], bf16, tag="x6", name="x6")
                xv = x5[0:52, :].rearrange("p (i w) -> p i w", w=4)
                ps5t = pp3.tile([104, 4 * GI * 8], f32, tag="ps3", name="ps5t")
                ps5 = ps5t[0:26, 0:256]
                pv = ps5.rearrange("p (i w) -> p i w", w=2)
                T.matmul(pv, tcons["bands5"][0:52, 26:52], xv[:, :, 0:4:2],
                         start=True, stop=False)
                T.matmul(pv, tcons["bands5"][0:52, 52:78], xv[:, :, 1:4:2],
                         start=False, stop=False)
                T.matmul(pv[:, :, 1:2], tcons["bands5"][0:52, 0:26],
                         xv[:, :, 1:2:2], start=False, stop=True,
                         skip_group_check=True)
                nc.scalar.activation(x6[:, :], ps5[:, :], AF.Relu,
                                     bias=tcons["bv5"][0:26, 0:1])

                # ---- conv6 (2x2 valid) -> t_u rows 0:13 ----
                xv = x6[0:26, :].rearrange("p (i w) -> p i w", w=2)
                ps6t = pp3.tile([104, 4 * GI * 8], f32, tag="ps3", name="ps6t")
                ps6 = ps6t[0:13, 0:128]
                T.matmul(ps6, tcons["bands6"][0:26, 0:13],
                         xv[:, :, 0:1], start=True, stop=False)
                T.matmul(ps6, tcons["bands6"][0:26, 13:26],
                         xv[:, :, 1:2], start=False, stop=True)
                nc.scalar.activation(t_u[0:13, b4 * 128:(b4 + 1) * 128], ps6,
                                     AF.Identity, bias=tcons["ubias"][0:13, 0:1],
                                     scale=2.5)

                # ---- KAN head for this half, interleaved with the other
                # half's conv stream ----
                kan_half(b4)

    nc.compile()
    _BUILT = (nc, ["xc"] + list(cons_shapes.keys()))
    return _BUILT


# ----------------------------------------------------------------------------
# entry point
# ----------------------------------------------------------------------------

def kernel(**inputs):
    from concourse import bass_utils

    x = np.asarray(inputs["x"], np.float32)
    cons = _host_consts({k: np.asarray(v, np.float32)
                         for k, v in inputs.items() if k != "x"})
    nc, _names = _build()

    in_maps = []
    for core in range(NCORE):
        xc = _shard_x(x[core * B:(core + 1) * B])
        in_maps.append({"xc": xc, **cons})
    res = bass_utils.run_bass_kernel_spmd(nc, in_maps, core_ids=list(range(NCORE)))
    return np.concatenate([r["out"] for r in res.results], axis=0)


# revision 4
# speedup vs baseline: 1.3414x; 1.3414x over previous
"""Trainium2 Bass kernel for nn_ChannelWiseConv (depthwise conv stack + KAN head).

v2 vs v1 (same conv1 front end, which already runs at the dispatch floor):
  - All post-conv bias+relu ops on the Activation engine only (each x2/x3/x5
    tile has a single writer engine -> no multi-writer wait-splitting).
  - conv4/5/6 and KAN PSUM tiles double-buffered so the two batch halves and
    consecutive KAN stages overlap instead of serializing on one bank.
  - KAN head: the D matrix (u - k replicated over basis offsets) is built by
    ONE PE matmul against a host-packed selector (replaces 12 DVE ops + a
    transpose per half), degree-0 basis via an is_ge difference (indicator
    subtraction), Cox-de-Boor levels entirely on DVE (gpsimd removed from
    the critical path), layer-2 silu taken from the SBUF copy of u2 so each
    PSUM tile keeps a single reader.
  - Cross-engine handoffs cost ~1.5-3us each on this relay when exposed
    (measured with synthetic probes); same-engine chains are ~free. The
    changes above cut the exposed-handoff count in the back half.
"""

import numpy as np

IN_CH, HIDDEN, NCLS = 13, 20, 10
B_FULL, NCORE = 2048, 8
B = B_FULL // NCORE          # images per core
NG = 16                      # image groups per core
GI = B // NG                 # images per group (16)
H_GRID = 0.4                 # KAN knot spacing; u = (x + 2.2) / 0.4

CG1 = [(0, 2), (2, 2), (4, 2), (6, 2), (8, 2), (10, 2), (12, 1)]
CG2 = [(0, 4), (4, 4), (8, 4), (12, 1)]
CG3 = [(0, 8), (8, 5)]

_BUILT = None  # cached (nc, input_names)


# ----------------------------------------------------------------------------
# host-side constant packing
# ----------------------------------------------------------------------------

def _pack_bands(w, S, cgs, slotM, rows):
    So = S // 2
    out = np.zeros((rows, len(cgs) * 3 * slotM), np.float32)
    for gi, (c0, nch) in enumerate(cgs):
        for b in range(3):
            col0 = (gi * 3 + b) * slotM
            for cl in range(nch):
                for i in range(So):
                    for a in range(3):
                        r = 2 * i + a - 1
                        if 0 <= r < S:
                            out[cl * S + r, col0 + cl * So + i] = w[c0 + cl, 0, a, b]
    return out


def _pack_bias_merged(bias, cgs, So, rows, per_col):
    ncol = (len(cgs) + per_col - 1) // per_col
    out = np.zeros((rows, ncol), np.float32)
    for gi, (c0, nch) in enumerate(cgs):
        col = gi // per_col
        p0 = (gi % per_col) * (cgs[0][1] * So)
        for cl in range(nch):
            out[p0 + cl * So:p0 + (cl + 1) * So, col] = bias[c0 + cl]
    return out


def _sel_D(nch):
    """Selector [nch+1, 12*nch]: col (k*nch+ch) -> row ch = 1, row nch = -k,
    so D = [u; 1].T @ S gives D[img, k*nch+ch] = u[ch, img] - k."""
    S = np.zeros((nch + 1, 12 * nch), np.float32)
    for k in range(12):
        for ch in range(nch):
            S[ch, k * nch + ch] = 1.0
            S[nch, k * nch + ch] = -float(k)
    return S


def _host_consts(inp):
    import ml_dtypes
    bf16 = ml_dtypes.bfloat16

    c = {}
    c["bands1"] = _pack_bands(inp["w1"], 64, CG1, 64, 128).astype(bf16)
    c["bands2"] = _pack_bands(inp["w2"], 32, CG2, 64, 128).astype(bf16)
    c["bands3"] = _pack_bands(inp["w3"], 16, CG3, 64, 128).astype(bf16)
    c["bands4"] = _pack_bands(inp["w4"], 8, [(0, 13)], 52, 104).astype(bf16)
    c["bands5"] = _pack_bands(inp["w5"], 4, [(0, 13)], 26, 52).astype(bf16)
    b6m = np.zeros((26, 26), np.float32)
    for bb in range(2):
        for ch in range(13):
            for r in range(2):
                b6m[ch * 2 + r, bb * 13 + ch] = inp["w6"][ch, 0, r, bb]
    c["bands6"] = b6m.astype(bf16)
    c["bv1"] = _pack_bias_merged(inp["b1"], CG1, 32, 128, 2)   # [128, 4]
    c["bv2"] = _pack_bias_merged(inp["b2"], CG2, 16, 128, 2)   # [128, 2]
    c["bv3"] = _pack_bias_merged(inp["b3"], CG3, 8, 104, 2)    # [104, 1]
    c["bv4"] = _pack_bias_merged(inp["b4"], [(0, 13)], 4, 52, 1)
    c["bv5"] = _pack_bias_merged(inp["b5"], [(0, 13)], 2, 26, 1)
    c["ubias"] = (2.5 * (inp["b6"] + 2.2)).astype(np.float32).reshape(13, 1)

    c["SD1"] = _sel_D(IN_CH)      # [14, 156]
    c["SD2"] = _sel_D(HIDDEN)     # [21, 240]

    # KAN layer 1 rhs pieces: silu part [13,20]; basis+bias part [105,20]
    c["c1a"] = np.ascontiguousarray(inp["sb1"].astype(np.float32))
    c1b = np.zeros((105, HIDDEN), np.float32)
    for n in range(8):
        for i in range(13):
            c1b[n * 13 + i] = inp["coef1"][i, :, n] * inp["ss1"][i] / 6.0
    c1b[104] = inp["bias1"]
    c["c1b"] = c1b
    # KAN layer 2 rhs pieces: silu [20,10]; basis rows 0..107; rows 108..159+bias
    c["c2s"] = np.ascontiguousarray(inp["sb2"].astype(np.float32))
    c2b = np.zeros((161, NCLS), np.float32)
    for n in range(8):
        for i in range(20):
            c2b[n * 20 + i] = inp["coef2"][i, :, n] * inp["ss2"][i] / 6.0
    c2b[160] = inp["bias2"]
    c["c2b1"] = np.ascontiguousarray(c2b[0:108])
    c["c2b2"] = np.ascontiguousarray(c2b[108:161])
    misc = np.zeros((128, 3), np.float32)
    misc[:, 0] = -2.2
    misc[:, 1] = 5.5
    c["misc"] = misc
    c["iden"] = np.eye(128, dtype=np.float32)
    return c


def _shard_x(x_shard):
    import ml_dtypes
    xs = x_shard.reshape(NG, GI, 13, 64, 64)
    xc = np.zeros((NG, 128, 7, GI, 64), np.float32)
    xa = xs[:, :, 0:12].transpose(0, 2, 3, 1, 4).reshape(NG, 6, 128, GI, 64)
    for cg in range(6):
        xc[:, :, cg] = xa[:, cg]
    xc[:, 0:64, 6] = xs[:, :, 12].transpose(0, 2, 1, 3)
    return np.ascontiguousarray(xc.astype(ml_dtypes.bfloat16))


# ----------------------------------------------------------------------------
# bass program
# ----------------------------------------------------------------------------

def _build():
    global _BUILT
    if _BUILT is not None:
        return _BUILT
    from contextlib import ExitStack
    import concourse.bass as bass  # noqa: F401
    import concourse.bacc as bacc
    import concourse.tile as tile
    import concourse.mybir as mybir

    f32 = mybir.dt.float32
    bf16 = mybir.dt.bfloat16
    AF = mybir.ActivationFunctionType
    OP = mybir.AluOpType
    AX = mybir.AxisListType

    nc = bacc.Bacc("TRN2")
    T = nc.tensor

    d_xc = nc.dram_tensor("xc", [NG, 128, 7, GI, 64], bf16, kind="ExternalInput")
    cons_shapes = {
        "bands1": ([128, 21 * 64], bf16), "bands2": ([128, 12 * 64], bf16),
        "bands3": ([128, 6 * 64], bf16), "bands4": ([104, 3 * 52], bf16),
        "bands5": ([52, 3 * 26], bf16), "bands6": ([26, 26], bf16),
        "bv1": ([128, 4], f32), "bv2": ([128, 2], f32), "bv3": ([104, 1], f32),
        "bv4": ([52, 1], f32), "bv5": ([26, 1], f32), "ubias": ([13, 1], f32),
        "misc": ([128, 3], f32), "c1a": ([13, 20], f32), "c1b": ([105, 20], f32),
        "c2s": ([20, 10], f32), "c2b1": ([108, 10], f32), "c2b2": ([53, 10], f32),
        "iden": ([128, 128], f32),
        "SD1": ([14, 156], f32), "SD2": ([21, 240], f32),
    }
    d_cons = {k: nc.dram_tensor(k, shp, dt, kind="ExternalInput")
              for k, (shp, dt) in cons_shapes.items()}
    d_out = nc.dram_tensor("out", [B, NCLS], f32, kind="ExternalOutput")

    with tile.TileContext(nc) as tc, ExitStack() as ctx:
        cpool = ctx.enter_context(tc.tile_pool(name="consts", bufs=1))
        tcons = {}
        early = ("bands1", "bands2", "bands3", "bv1", "bv2", "bv3")
        for k, (shp, dt) in cons_shapes.items():
            t = cpool.tile(shp, dt, name=f"t_{k}")
            if k in early:
                nc.scalar.dma_start(t[:, :], d_cons[k][:, :])
            else:
                nc.gpsimd.dma_start(t[:, :], d_cons[k][:, :])
            tcons[k] = t
        t_u = cpool.tile([14, B], f32, name="t_u")
        nc.vector.memset(t_u[:, :], 1.0)   # row 13 stays 1; rows 0:13 overwritten

        p_x1 = ctx.enter_context(tc.tile_pool(name="x1", bufs=3))
        p_x2 = ctx.enter_context(tc.tile_pool(name="x2", bufs=2))
        p_x3 = ctx.enter_context(tc.tile_pool(name="x3", bufs=2))
        p_sm = ctx.enter_context(tc.tile_pool(name="xsm", bufs=2))

        with tc.tile_pool(name="ps1", bufs=2, space="PSUM") as pp1, \
             tc.tile_pool(name="ps2", bufs=2, space="PSUM") as pp2, \
             tc.tile_pool(name="ps3456", bufs=2, space="PSUM") as pp3, \
             tc.tile_pool(name="kanps", bufs=1, space="PSUM") as ppk:
            scrap = pp1.tile([1, 16], f32, tag="scrap", bufs=1, name="scrap")
            for k in ("bands1", "bands2", "bands3"):
                T.matmul(scrap[0:1, 0:1], tcons[k][0:1, 0:1],
                         tcons[k][0:1, 0:1], start=True, stop=True)
            kpool = ctx.enter_context(tc.tile_pool(name="kan", bufs=2))

            def kan_half(t):
                sl = slice(t * 128, (t + 1) * 128)
                # ---- D = u - k for all (k, ch) via one matmul ----
                psD = ppk.tile([128, 256], f32, tag="kan", name="psD")
                T.matmul(psD[0:128, 0:156], t_u[0:14, sl], tcons["SD1"][:, :],
                         start=True, stop=True)
                D = kpool.tile([128, 156], f32, tag="D", name="Dt")
                nc.vector.tensor_copy(D[:, :], psD[0:128, 0:156])
                ge = kpool.tile([128, 156], f32, tag="ge", name="ge")
                nc.vector.tensor_scalar(ge[:, :], D[:, :], 0.0, None,
                                        op0=OP.is_ge)
                Bc = kpool.tile([128, 143], f32, tag="B0", name="Bc")
                nc.vector.tensor_sub(Bc[:, :], ge[:, 0:143], ge[:, 13:156])
                wid = 143
                for p in range(1, 4):
                    wid -= 13
                    ta = kpool.tile([128, wid], f32, tag=f"ta{p}", name="ta")
                    tb = kpool.tile([128, wid], f32, tag=f"tb{p}", name="tb")
                    nc.vector.tensor_mul(ta[:, :], D[:, 0:wid], Bc[:, 0:wid])
                    nc.vector.tensor_mul(tb[:, :],
                                         D[:, 13 * (p + 1):13 * (p + 1) + wid],
                                         Bc[:, 13:13 + wid])
                    if p < 3:
                        Bc = kpool.tile([128, wid], f32, tag=f"B{p}", name="Bc")
                        nc.vector.tensor_sub(Bc[:, :], ta[:, :], tb[:, :])
                    else:
                        Bc = kpool.tile([128, 105], f32, tag="B3", name="Bc")
                        nc.vector.tensor_sub(Bc[:, 0:104], ta[:, :], tb[:, :])
                        nc.vector.memset(Bc[:, 104:105], 1.0)
                # stacks: silu part [13,128]; (basis;1)^T part [105,128]
                stkA = kpool.tile([13, 128], f32, tag="stkA", name="stkA")
                stkB = kpool.tile([105, 128], f32, tag="stkB", name="stkB")
                ps_b1 = ppk.tile([128, 256], f32, tag="kan", name="ps_b1")
                T.transpose(ps_b1[0:105, 0:128], Bc[:, 0:105],
                            tcons["iden"][:, :])
                nc.vector.tensor_copy(stkB[:, :], ps_b1[0:105, 0:128])
                nc.scalar.activation(stkA[:, :], t_u[0:13, sl], AF.Silu,
                                     bias=tcons["misc"][0:13, 0:1], scale=H_GRID)
                ps_h1 = ppk.tile([128, 256], f32, tag="kan", name="ps_h1")
                T.matmul(ps_h1[0:128, 0:20], stkA[:, :], tcons["c1a"][:, :],
                         start=True, stop=False)
                T.matmul(ps_h1[0:128, 0:20], stkB[:, :], tcons["c1b"][:, :],
                         start=False, stop=True)
                # ---- KAN layer 2 ----
                u2 = kpool.tile([128, 20], f32, tag="u2", name="u2")
                nc.scalar.activation(u2[:, :], ps_h1[0:128, 0:20], AF.Identity,
                                     bias=tcons["misc"][0:128, 1:2], scale=2.5)
                # u2T with a ones row: memset whole, overwrite rows 0:20
                u2t = kpool.tile([21, 128], f32, tag="u2t", name="u2t")
                nc.vector.memset(u2t[:, :], 1.0)
                ps_t2 = ppk.tile([128, 256], f32, tag="kan", name="ps_t2")
                T.transpose(ps_t2[0:20, 0:128], u2[:, :], tcons["iden"][:, :])
                nc.vector.tensor_copy(u2t[0:20, :], ps_t2[0:20, 0:128])
                stk2s = kpool.tile([20, 128], f32, tag="s2s", name="stk2s")
                nc.scalar.activation(stk2s[:, :], u2t[0:20, :], AF.Silu,
                                     bias=tcons["misc"][0:20, 0:1], scale=H_GRID)
                psD2 = ppk.tile([128, 256], f32, tag="kan", name="psD2")
                T.matmul(psD2[0:128, 0:240], u2t[0:21, :], tcons["SD2"][:, :],
                         start=True, stop=True)
                D2 = kpool.tile([128, 240], f32, tag="D2", name="D2t")
                nc.vector.tensor_copy(D2[:, :], psD2[0:128, 0:240])
                ge2 = kpool.tile([128, 240], f32, tag="ge2", name="ge2")
                nc.vector.tensor_scalar(ge2[:, :], D2[:, :], 0.0, None,
                                        op0=OP.is_ge)
                Bc2 = kpool.tile([128, 220], f32, tag="B0_2", name="Bc2")
                nc.vector.tensor_sub(Bc2[:, :], ge2[:, 0:220], ge2[:, 20:240])
                wid = 220
                for p in range(1, 4):
                    wid -= 20
                    ta = kpool.tile([128, wid], f32, tag=f"t2a{p}", name="ta2")
                    tb = kpool.tile([128, wid], f32, tag=f"t2b{p}", name="tb2")
                    nc.vector.tensor_mul(ta[:, :], D2[:, 0:wid], Bc2[:, 0:wid])
                    nc.vector.tensor_mul(tb[:, :],
                                         D2[:, 20 * (p + 1):20 * (p + 1) + wid],
                                         Bc2[:, 20:20 + wid])
                    if p < 3:
                        Bc2 = kpool.tile([128, wid], f32, tag=f"B{p}_2",
                                         name="Bc2")
                        nc.vector.tensor_sub(Bc2[:, :], ta[:, :], tb[:, :])
                    else:
                        Bc2 = kpool.tile([128, 161], f32, tag="B3_2",
                                         name="Bc2")
                        nc.vector.tensor_sub(Bc2[:, 0:160], ta[:, :], tb[:, :])
                        nc.vector.memset(Bc2[:, 160:161], 1.0)
                stk2a = kpool.tile([108, 128], f32, tag="s2a", name="stk2a")
                stk2b = kpool.tile([53, 128], f32, tag="s2b", name="stk2b")
                ps_b2 = ppk.tile([128, 256], f32, tag="kan", name="ps_b2")
                T.transpose(ps_b2[0:108, 0:128], Bc2[:, 0:108],
                            tcons["iden"][:, :])
                nc.vector.tensor_copy(stk2a[:, :], ps_b2[0:108, 0:128])
                ps_b3 = ppk.tile([128, 256], f32, tag="kan", name="ps_b3")
                T.transpose(ps_b3[0:53, 0:128], Bc2[:, 108:161],
                            tcons["iden"][:, :])
                nc.vector.tensor_copy(stk2b[:, :], ps_b3[0:53, 0:128])
                ps_lg = ppk.tile([128, 256], f32, tag="kan", name="ps_lg")
                T.matmul(ps_lg[0:128, 0:NCLS], stk2a[:, :], tcons["c2b1"][:, :],
                         start=True, stop=False)
                T.matmul(ps_lg[0:128, 0:NCLS], stk2s[:, :], tcons["c2s"][:, :],
                         start=False, stop=False)
                T.matmul(ps_lg[0:128, 0:NCLS], stk2b[:, :], tcons["c2b2"][:, :],
                         start=False, stop=True)
                # ---- log_softmax (on an SBUF copy; ps_lg keeps 1 reader) ----
                lg_s = kpool.tile([128, NCLS], f32, tag="lg_s", name="lg_s")
                nc.vector.tensor_copy(lg_s[:, :], ps_lg[0:128, 0:NCLS])
                negm = kpool.tile([128, 1], f32, tag="negm", name="negm")
                nc.vector.reduce_max(negm[:, :], lg_s[:, :], axis=AX.X,
                                     negate=True)
                ex = kpool.tile([128, NCLS], f32, tag="ex", name="ex")
                nc.scalar.activation(ex[:, :], lg_s[:, :], AF.Exp,
                                     bias=negm[:, 0:1])
                ssum = kpool.tile([128, 1], f32, tag="ssum", name="ssum")
                nc.vector.reduce_sum(ssum[:, :], ex[:, :], axis=AX.X)
                lsum = kpool.tile([128, 1], f32, tag="lsum", name="lsum")
                nc.scalar.activation(lsum[:, :], ssum[:, :], AF.Ln,
                                     bias=tcons["misc"][0:128, 2:3])
                res = kpool.tile([128, NCLS], f32, tag="res", name="res")
                nc.vector.tensor_scalar(res[:, :], lg_s[:, :],
                                        negm[:, 0:1], lsum[:, 0:1],
                                        op0=OP.add, op1=OP.subtract)
                nc.sync.dma_start(d_out[sl, :], res[:, :])

            x2t = x3t = x4 = x5 = None
            for g in range(NG):
                xt = p_x1.tile([128, 7 * GI * 64], bf16, tag="x1", name="xt")
                nc.sync.dma_start(
                    xt[:, :].rearrange("p (s i w) -> p s i w", i=GI, w=64),
                    d_xc[g, :, :, :, :])
                T.matmul(scrap[0:1, 0:1], xt[0:1, 0:1], xt[0:1, 0:1],
                         start=True, stop=True)
                # ---- conv1 ----
                if g % 2 == 0:
                    x2t = [p_x2.tile([128, 2 * GI * 32], bf16, tag=f"x2_{k}",
                                     name=f"x2_{k}") for k in range(4)]
                c0_, c1_ = (g % 2) * GI * 32, (g % 2 + 1) * GI * 32
                ps_pair = None
                for cg, (c0, nch) in enumerate(CG1):
                    K, M = nch * 64, nch * 32
                    xv = xt[0:K, cg * GI * 64:(cg + 1) * GI * 64].rearrange(
                        "p (i w) -> p i w", w=64)
                    if cg % 2 == 0:
                        ps_pair = pp1.tile([128, GI * 32], f32, tag="ps1",
                                           name="ps1t")
                    po = 64 * (cg % 2)
                    ps = ps_pair[po:po + M, :]
                    pv = ps.rearrange("p (i w) -> p i w", w=32)
                    lo = lambda b: (cg * 3 + b) * 64
                    T.matmul(pv, tcons["bands1"][0:K, lo(1):lo(1) + M],
                             xv[:, :, 0:64:2], start=True, stop=False,
                             skip_group_check=True)
                    T.matmul(pv, tcons["bands1"][0:K, lo(2):lo(2) + M],
                             xv[:, :, 1:64:2], start=False, stop=False,
                             skip_group_check=True)
                    T.matmul(pv[:, :, 1:32], tcons["bands1"][0:K, lo(0):lo(0) + M],
                             xv[:, :, 1:62:2], start=False, stop=True,
                             skip_group_check=True)
                    if cg % 2 == 1 or cg == 6:
                        pair = cg // 2
                        Mt = 128 if cg != 6 else 32
                        nc.scalar.activation(x2t[pair][0:Mt, c0_:c1_],
                                             ps_pair[0:Mt, :], AF.Relu,
                                             bias=tcons["bv1"][0:Mt, pair:pair + 1])
                if g == 1:
                    for k in ("bands4", "bands5", "bands6", "c1a", "c1b",
                              "c2s", "c2b1", "c2b2", "iden", "SD1", "SD2",
                              "misc"):
                        T.matmul(scrap[0:1, 0:1], tcons[k][0:1, 0:1],
                                 tcons[k][0:1, 0:1], start=True, stop=True)
                if g % 2 != 1:
                    continue

                # ---- conv2 over a 2-group batch ----
                b2 = g // 2
                if b2 % 2 == 0:
                    x3t = [p_x3.tile([128, 4 * GI * 16], bf16, tag=f"x3_{k}",
                                     name=f"x3_{k}") for k in range(2)]
                d0, d1 = (b2 % 2) * 512, (b2 % 2 + 1) * 512
                ps_pair = None
                for k4, (c0, nch) in enumerate(CG2):
                    K, M = nch * 32, nch * 16
                    xv = x2t[k4][0:K, :].rearrange("p (i w) -> p i w", w=32)
                    if k4 % 2 == 0:
                        ps_pair = pp2.tile([128, 2 * GI * 16], f32, tag="ps2",
                                           name="ps2t")
                    po = 64 * (k4 % 2)
                    ps = ps_pair[po:po + M, :]
                    pv = ps.rearrange("p (i w) -> p i w", w=16)
                    lo = lambda b: (k4 * 3 + b) * 64
                    T.matmul(pv, tcons["bands2"][0:K, lo(1):lo(1) + M],
                             xv[:, :, 0:32:2], start=True, stop=False,
                             skip_group_check=True)
                    T.matmul(pv, tcons["bands2"][0:K, lo(2):lo(2) + M],
                             xv[:, :, 1:32:2], start=False, stop=False,
                             skip_group_check=True)
                    T.matmul(pv[:, :, 1:16], tcons["bands2"][0:K, lo(0):lo(0) + M],
                             xv[:, :, 1:30:2], start=False, stop=True,
                             skip_group_check=True)
                    if k4 % 2 == 1:
                        pair = k4 // 2
                        Mt = 128 if pair == 0 else 80
                        nc.scalar.activation(x3t[pair][0:Mt, d0:d1],
                                             ps_pair[0:Mt, :], AF.Relu,
                                             bias=tcons["bv2"][0:Mt, pair:pair + 1])
                if b2 % 2 != 1:
                    continue

                # ---- conv3 over a 4-group batch ----
                b3 = b2 // 2
                if b3 % 2 == 0:
                    x4 = p_sm.tile([104, 8 * GI * 8], bf16, tag="x4", name="x4")
                e0, e1 = (b3 % 2) * 512, (b3 % 2 + 1) * 512
                ps3 = pp3.tile([104, 4 * GI * 8], f32, tag="ps3", name="ps3t")
                for k8, (c0, nch) in enumerate(CG3):
                    K, M = nch * 16, nch * 8
                    xv = x3t[k8][0:K, :].rearrange("p (i w) -> p i w", w=16)
                    ps = ps3[64 * k8:64 * k8 + M, :]
                    pv = ps.rearrange("p (i w) -> p i w", w=8)
                    lo = lambda b: (k8 * 3 + b) * 64
                    T.matmul(pv, tcons["bands3"][0:K, lo(1):lo(1) + M],
                             xv[:, :, 0:16:2], start=True, stop=False,
                             skip_group_check=True)
                    T.matmul(pv, tcons["bands3"][0:K, lo(2):lo(2) + M],
                             xv[:, :, 1:16:2], start=False, stop=False,
                             skip_group_check=True)
                    T.matmul(pv[:, :, 1:8], tcons["bands3"][0:K, lo(0):lo(0) + M],
                             xv[:, :, 1:14:2], start=False, stop=True,
                             skip_group_check=True)
                nc.scalar.activation(x4[:, e0:e1], ps3[0:104, :], AF.Relu,
                                     bias=tcons["bv3"][0:104, 0:1])
                if b3 % 2 != 1:
                    continue

                # ---- conv4 over an 8-group batch ----
                b4 = b3 // 2
                x5 = p_sm.tile([52, 512], bf16, tag="x5", name="x5")
                xv = x4[0:104, :].rearrange("p (i w) -> p i w", w=8)
                ps4t = pp3.tile([104, 4 * GI * 8], f32, tag="ps3", name="ps4t")
                ps4 = ps4t[0:52, :]
                pv = ps4.rearrange("p (i w) -> p i w", w=4)
                T.matmul(pv, tcons["bands4"][0:104, 52:104], xv[:, :, 0:8:2],
                         start=True, stop=False)
                T.matmul(pv, tcons["bands4"][0:104, 104:156], xv[:, :, 1:8:2],
                         start=False, stop=False)
                T.matmul(pv[:, :, 1:4], tcons["bands4"][0:104, 0:52],
                         xv[:, :, 1:6:2], start=False, stop=True,
                         skip_group_check=True)
                nc.scalar.activation(x5[:, :], ps4[:, :], AF.Relu,
                                     bias=tcons["bv4"][0:52, 0:1])

                # ---- conv5 ----
                x6 = p_sm.tile([26, 256], bf16, tag="x6", name="x6")
                xv = x5[0:52, :].rearrange("p (i w) -> p i w", w=4)
                ps5t = pp3.tile([104, 4 * GI * 8], f32, tag="ps3", name="ps5t")
                ps5 = ps5t[0:26, 0:256]
                pv = ps5.rearrange("p (i w) -> p i w", w=2)
                T.matmul(pv, tcons["bands5"][0:52, 26:52], xv[:, :, 0:4:2],
                         start=True, stop=False)
                T.matmul(pv, tcons["bands5"][0:52, 52:78], xv[:, :, 1:4:2],
                         start=False, stop=False)
                T.matmul(pv[:, :, 1:2], tcons["bands5"][0:52, 0:26],
                         xv[:, :, 1:2:2], start=False, stop=True,
                         skip_group_check=True)
                nc.scalar.activation(x6[:, :], ps5[:, :], AF.Relu,
                                     bias=tcons["bv5"][0:26, 0:1])

                # ---- conv6 (2x2 valid) -> t_u rows 0:13 ----
                xv = x6[0:26, :].rearrange("p (i w) -> p i w", w=2)
                ps6t = pp3.tile([104, 4 * GI * 8], f32, tag="ps3", name="ps6t")
                ps6 = ps6t[0:13, 0:128]
                T.matmul(ps6, tcons["bands6"][0:26, 0:13],
                         xv[:, :, 0:1], start=True, stop=False)
                T.matmul(ps6, tcons["bands6"][0:26, 13:26],
                         xv[:, :, 1:2], start=False, stop=True)
                nc.scalar.activation(t_u[0:13, b4 * 128:(b4 + 1) * 128], ps6,
                                     AF.Identity, bias=tcons["ubias"][0:13, 0:1],
                                     scale=2.5)

                # ---- KAN head for this half, interleaved with the other
                # half's conv stream ----
                kan_half(b4)

    nc.compile()
    _BUILT = (nc, ["xc"] + list(cons_shapes.keys()))
    return _BUILT


# ----------------------------------------------------------------------------
# entry point
# ----------------------------------------------------------------------------

def kernel(**inputs):
    from concourse import bass_utils

    x = np.asarray(inputs["x"], np.float32)
    cons = _host_consts({k: np.asarray(v, np.float32)
                         for k, v in inputs.items() if k != "x"})
    nc, _names = _build()

    in_maps = []
    for core in range(NCORE):
        xc = _shard_x(x[core * B:(core + 1) * B])
        in_maps.append({"xc": xc, **cons})
    res = bass_utils.run_bass_kernel_spmd(nc, in_maps, core_ids=list(range(NCORE)))
    return np.concatenate([r["out"] for r in res.results], axis=0)
